# revision 13
# baseline (speedup 1.0000x reference)
"""Trainium2 Bass kernel for the Metric GNN message-passing problem.

Math (matches reference.py):
    h      = relu(features @ W1 + b1)
    metric = (h @ W2 + b2).reshape(N, 3, 3)          # per-vertex 3x3
    For each edge e=(s,d):
        t      = vertices[d] - vertices[s]
        dist2  = || metric[s] @ t ||^2  = t' G t,  G = metric' metric (sym 3x3)
        w      = 1 / (1 + dist2)
    row_sum = segment_sum(w, src);  col_sum = segment_sum(w, dst)
    values  = 0.5 * concat(w / row_sum[src], w / col_sum[dst])
    indices = [[src, dst], [dst, src]]  (pure rearrangement of the input edges)

Distribution over 8 NeuronCores (one SPMD program):
  - Edges are partitioned into 8 contiguous chunks (edge parallelism).
  - MLP params replicated; each core computes the metric/G table only for its
    1/8 node range, then the per-node table T = [G(6 floats, off-diag doubled),
    v(3), pad(3)] is AllGather'd so every core can gather rows for its edges.
  - Per-edge gathers T[src], v[dst] via indirect DMA.
  - Per-node segment sums: each edge's w is scattered (plain write, no RMW)
    into a private DRAM "spread" table at (node*S + occ) where occ is the
    edge's occurrence rank within its node for this core's chunk (a
    collision-free slot layout precomputed on the host as part of sharding);
    the table is then dense-reduced over the S slots to per-node partial sums,
    which are AllReduce'd across the 8 cores (degree normalization).
  - Pass 2 gathers the per-node reciprocals and scales w.
"""

import sys

for _p in ("/opt/trn_rl_repo", "/root/.axon_site/_ro/trn_rl_repo"):
    if _p not in sys.path:
        sys.path.append(_p)

import numpy as np

import concourse.bass as bass
import concourse.mybir as mybir
from concourse import bacc
from concourse.masks import make_identity
from concourse.tile import TileContext

F32 = mybir.dt.float32
I32 = mybir.dt.int32
AX = mybir.AxisListType
ALU = mybir.AluOpType
ACT_FN = mybir.ActivationFunctionType

NCORES = 8

FULL_CFG = dict(
    N=100000,       # vertices
    E=3200000,      # edges
    F_DIM=64,
    H_DIM=32,
    S=24,           # spread slots per node (max observed per-chunk degree 17)
    EJ=128,         # free-dim width of an edge tile -> tile = 16384 edges
    RC=8,           # reduce chunks over the spread table
)


def derive(cfg):
    c = dict(cfg)
    N, E = c["N"], c["E"]
    assert N % NCORES == 0 and E % NCORES == 0
    c["NSH"] = N // NCORES                      # nodes per core (MLP shard)
    c["EPC"] = E // NCORES                      # edges per core
    c["TILE_E"] = 128 * c["EJ"]
    c["NT"] = -(-c["EPC"] // c["TILE_E"])       # edge tiles per core
    c["EPAD"] = c["NT"] * c["TILE_E"]           # padded edges per core
    c["MF"] = c["NSH"] // 128                   # full 128-node MLP subtiles
    c["MR"] = c["NSH"] % 128                    # remainder subtile size
    # padded node count for the sum tables; must be divisible by RC*128 and > N
    rc128 = c["RC"] * 128
    c["NPAD"] = -(-(N + 1) // rc128) * rc128
    c["RS"] = c["NPAD"] // rc128                # free-dim nodes per reduce chunk
    # zeroing of the spread tables: NZ dma chunks of [128, ZC]
    sp = c["NPAD"] * c["S"]
    assert sp % 128 == 0
    per_part = sp // 128
    nz = 16
    while per_part % nz:
        nz -= 1
    c["NZ"], c["ZC"] = nz, per_part // nz
    return c


def build_kernel(cfg, debug=False):
    """Builds the single SPMD Bass program run on all 8 cores."""
    c = derive(cfg)
    N, F_DIM, H_DIM = c["N"], c["F_DIM"], c["H_DIM"]
    NSH, EPAD, EJ, NT = c["NSH"], c["EPAD"], c["EJ"], c["NT"]
    S, NPAD, RC, RS = c["S"], c["NPAD"], c["RC"], c["RS"]
    MF, MR = c["MF"], c["MR"]
    MSUB = MF + (1 if MR else 0)

    nc = bacc.Bacc(None, num_devices=NCORES)

    # ---- I/O ----
    feat = nc.declare_dram_parameter("feat", [NSH, F_DIM], F32, isOutput=False)
    vert = nc.declare_dram_parameter("vert", [NSH, 3], F32, isOutput=False)
    W1 = nc.declare_dram_parameter("W1", [F_DIM, H_DIM], F32, isOutput=False)
    b1 = nc.declare_dram_parameter("b1", [H_DIM], F32, isOutput=False)
    W2 = nc.declare_dram_parameter("W2", [H_DIM, 9], F32, isOutput=False)
    b2 = nc.declare_dram_parameter("b2", [9], F32, isOutput=False)
    src = nc.declare_dram_parameter("src", [EPAD], I32, isOutput=False)
    dst = nc.declare_dram_parameter("dst", [EPAD], I32, isOutput=False)
    a_row = nc.declare_dram_parameter("a_row", [EPAD], I32, isOutput=False)
    a_col = nc.declare_dram_parameter("a_col", [EPAD], I32, isOutput=False)
    w_row = nc.declare_dram_parameter("w_row", [EPAD], F32, isOutput=True)
    w_col = nc.declare_dram_parameter("w_col", [EPAD], F32, isOutput=True)
    if debug:
        dbg_T = nc.declare_dram_parameter("dbg_T", [256, 12], F32, isOutput=True)
        dbg_w = nc.declare_dram_parameter("dbg_w", [c["TILE_E"]], F32, isOutput=True)
        dbg_ps = nc.declare_dram_parameter("dbg_ps", [2, 256], F32, isOutput=True)
        dbg_po = nc.declare_dram_parameter("dbg_po", [2, 256], F32, isOutput=True)
        dbg_r = nc.declare_dram_parameter("dbg_r", [256], F32, isOutput=True)

    # ---- internal DRAM ----
    T_sh = nc.dram_tensor("T_sh", [NSH, 12], F32)
    T_full = nc.dram_tensor("T_full", [N, 12], F32, addr_space="Shared")
    spread_r = nc.dram_tensor("spread_r", [NPAD * S], F32)
    spread_c = nc.dram_tensor("spread_c", [NPAD * S], F32)
    psum_in = nc.dram_tensor("psum_in", [2, NPAD], F32)
    psum_out = nc.dram_tensor("psum_out", [2, NPAD], F32, addr_space="Shared")
    r_row = nc.dram_tensor("r_row", [NPAD], F32)
    r_col = nc.dram_tensor("r_col", [NPAD], F32)

    groups = [list(range(NCORES))]

    with TileContext(nc) as tc:
        with (
            tc.tile_pool(name="const", bufs=1) as constp,
            tc.tile_pool(name="persist", bufs=1) as persist,
            tc.tile_pool(name="mlp", bufs=3) as mlp,
            tc.tile_pool(name="mlp_ps", bufs=2, space="PSUM") as mlp_ps,
            tc.tile_pool(name="edge", bufs=3) as edge,
            tc.tile_pool(name="gath", bufs=3) as gath,
            tc.tile_pool(name="work", bufs=3) as work,
            tc.tile_pool(name="red", bufs=3) as red,
        ):
            # ================= Phase Z: zero the spread tables ==============
            zt = constp.tile([128, c["ZC"]], F32, tag="zero")
            nc.vector.memset(zt[:], 0.0)
            for tbl in (spread_r, spread_c):
                v = tbl[:].rearrange("(z p q) -> z p q", z=c["NZ"], p=128)
                for i in range(c["NZ"]):
                    nc.sync.dma_start(out=v[i], in_=zt[:])

            # ================= Phase A: MLP -> T table =====================
            ident = constp.tile([128, 128], F32, tag="ident")
            make_identity(nc, ident[:])
            W1_sb = constp.tile([F_DIM, H_DIM], F32, tag="W1")
            nc.sync.dma_start(out=W1_sb[:], in_=W1[:, :])
            W2_sb = constp.tile([H_DIM, 9], F32, tag="W2")
            nc.sync.dma_start(out=W2_sb[:], in_=W2[:, :])
            b1_sb = constp.tile([H_DIM, 1], F32, tag="b1")
            nc.sync.dma_start(out=b1_sb[:], in_=b1[:, None])
            b2_sb = constp.tile([9, 1], F32, tag="b2")
            nc.sync.dma_start(out=b2_sb[:], in_=b2[:, None])

            # node layout: node = s*128 + p  (s = subtile, p = partition)
            mT = persist.tile([128, MSUB, 9], F32, tag="mT")  # metric, node-major
            for s in range(MSUB):
                P = 128 if s < MF else MR
                ft = mlp.tile([128, F_DIM], F32, tag="ft")
                nc.sync.dma_start(out=ft[:P], in_=feat[s * 128 : s * 128 + P, :])
                ftT_ps = mlp_ps.tile([F_DIM, 128], F32, tag="ftT")
                nc.tensor.transpose(out=ftT_ps[:, :P], in_=ft[:P], identity=ident[:P, :P])
                ftT = mlp.tile([F_DIM, 128], F32, tag="ftT_sb")
                nc.vector.tensor_copy(out=ftT[:, :P], in_=ftT_ps[:, :P])
                h_ps = mlp_ps.tile([H_DIM, 128], F32, tag="h_ps")
                nc.tensor.matmul(out=h_ps[:, :P], lhsT=W1_sb[:], rhs=ftT[:, :P],
                                 start=True, stop=True)
                h_sb = mlp.tile([H_DIM, 128], F32, tag="h_sb")
                nc.scalar.activation(out=h_sb[:, :P], in_=h_ps[:, :P],
                                     func=ACT_FN.Relu, bias=b1_sb[:])
                m_ps = mlp_ps.tile([9, 128], F32, tag="m_ps")
                nc.tensor.matmul(out=m_ps[:, :P], lhsT=W2_sb[:], rhs=h_sb[:, :P],
                                 start=True, stop=True)
                m_sb = mlp.tile([9, 128], F32, tag="m_sb")
                nc.vector.tensor_scalar(out=m_sb[:, :P], in0=m_ps[:, :P],
                                        scalar1=b2_sb[:], scalar2=None, op0=ALU.add)
                mT_ps = mlp_ps.tile([128, 9], F32, tag="mT_ps")
                nc.tensor.transpose(out=mT_ps[:P], in_=m_sb[:, :P], identity=ident[:9, :9])
                nc.vector.tensor_copy(out=mT[:P, s, :], in_=mT_ps[:P])

            # G = M' M (symmetric, off-diagonals doubled), node-major layout
            Tst = persist.tile([128, MSUB, 12], F32, tag="Tst")
            nc.vector.memset(Tst[:], 0.0)
            tmp = persist.tile([128, MSUB], F32, tag="gtmp")
            for g, (i, j) in enumerate([(0, 0), (1, 1), (2, 2), (0, 1), (0, 2), (1, 2)]):
                dstv = Tst[:, :, g]
                nc.vector.tensor_tensor(out=dstv, in0=mT[:, :, i], in1=mT[:, :, j],
                                        op=ALU.mult)
                for k in (1, 2):
                    nc.vector.tensor_tensor(out=tmp[:], in0=mT[:, :, 3 * k + i],
                                            in1=mT[:, :, 3 * k + j], op=ALU.mult)
                    nc.vector.tensor_tensor(out=dstv, in0=dstv, in1=tmp[:], op=ALU.add)
            # double the off-diagonal entries
            nc.vector.tensor_scalar(out=Tst[:, :, 3:6], in0=Tst[:, :, 3:6],
                                    scalar1=2.0, scalar2=None, op0=ALU.mult)
            # vertices into T cols 6:9
            vs = persist.tile([128, MSUB, 3], F32, tag="vs")
            if MF:
                nc.sync.dma_start(
                    out=vs[:, :MF, :],
                    in_=vert[: MF * 128, :].rearrange("(s p) k -> p s k", p=128))
            if MR:
                nc.sync.dma_start(out=vs[:MR, MF, :], in_=vert[MF * 128 :, :])
            nc.vector.tensor_copy(out=Tst[:, :, 6:9], in_=vs[:])

            # write T shard (node = s*128 + p ordering -> node-ordered rows)
            if MF:
                nc.sync.dma_start(
                    out=T_sh[: MF * 128, :].rearrange("(s p) k -> p s k", p=128),
                    in_=Tst[:, :MF, :])
            if MR:
                nc.sync.dma_start(out=T_sh[MF * 128 :, :], in_=Tst[:MR, MF, :])

            nc.gpsimd.collective_compute(
                "AllGather", ALU.bypass, replica_groups=groups,
                ins=[T_sh[:, :]], outs=[T_full[:, :]])

            # ================= Phase B: per-edge w + scatter ================
            w_all = persist.tile([128, NT * EJ], F32, tag="w_all")
            src_v = src[:].rearrange("(t p j) -> t p j", t=NT, p=128)
            dst_v = dst[:].rearrange("(t p j) -> t p j", t=NT, p=128)
            ar_v = a_row[:].rearrange("(t p j) -> t p j", t=NT, p=128)
            ac_v = a_col[:].rearrange("(t p j) -> t p j", t=NT, p=128)

            for t in range(NT):
                st = edge.tile([128, EJ], I32, tag="st")
                dt_ = edge.tile([128, EJ], I32, tag="dt")
                art = edge.tile([128, EJ], I32, tag="art")
                act = edge.tile([128, EJ], I32, tag="act")
                nc.sync.dma_start(out=st[:], in_=src_v[t])
                nc.sync.dma_start(out=dt_[:], in_=dst_v[t])
                nc.sync.dma_start(out=art[:], in_=ar_v[t])
                nc.sync.dma_start(out=act[:], in_=ac_v[t])

                # HW vector-indirect DMA consumes ONE index per partition per
                # instruction (payload contiguous from that offset), so gather
                # 128 rows per instruction, one column slice at a time.
                Ts = gath.tile([128, EJ, 12], F32, tag="Ts")
                for j in range(EJ):
                    nc.gpsimd.indirect_dma_start(
                        out=Ts[:, j, :], out_offset=None, in_=T_full[:, :],
                        in_offset=bass.IndirectOffsetOnAxis(ap=st[:, j : j + 1], axis=0))
                Vd = gath.tile([128, EJ, 3], F32, tag="Vd")
                for j in range(EJ):
                    nc.gpsimd.indirect_dma_start(
                        out=Vd[:, j, :], out_offset=None, in_=T_full[:, :],
                        in_offset=bass.IndirectOffsetOnAxis(ap=dt_[:, j : j + 1], axis=0),
                        element_offset=6)

                tan = work.tile([128, EJ, 3], F32, tag="tan")
                nc.vector.tensor_tensor(out=tan[:], in0=Vd[:], in1=Ts[:, :, 6:9],
                                        op=ALU.subtract)
                tx, ty, tz = tan[:, :, 0], tan[:, :, 1], tan[:, :, 2]
                q = work.tile([128, EJ], F32, tag="q")
                pr = work.tile([128, EJ], F32, tag="pr")
                pq = work.tile([128, EJ], F32, tag="pq")
                terms = [(tx, tx, 0), (ty, ty, 1), (tz, tz, 2),
                         (tx, ty, 3), (tx, tz, 4), (ty, tz, 5)]
                for n, (a, b, g) in enumerate(terms):
                    nc.vector.tensor_tensor(out=pr[:], in0=a, in1=b, op=ALU.mult)
                    if n == 0:
                        nc.vector.tensor_tensor(out=q[:], in0=pr[:], in1=Ts[:, :, g],
                                                op=ALU.mult)
                    else:
                        nc.vector.tensor_tensor(out=pq[:], in0=pr[:], in1=Ts[:, :, g],
                                                op=ALU.mult)
                        nc.vector.tensor_tensor(out=q[:], in0=q[:], in1=pq[:],
                                                op=ALU.add)
                nc.vector.tensor_scalar(out=q[:], in0=q[:], scalar1=1.0,
                                        scalar2=None, op0=ALU.add)
                wt = w_all[:, t * EJ : (t + 1) * EJ]
                nc.vector.reciprocal(out=wt, in_=q[:])

                for j in range(EJ):
                    nc.gpsimd.indirect_dma_start(
                        out=spread_r[:, None], in_=wt[:, j : j + 1], in_offset=None,
                        out_offset=bass.IndirectOffsetOnAxis(ap=art[:, j : j + 1], axis=0))
                for j in range(EJ):
                    nc.gpsimd.indirect_dma_start(
                        out=spread_c[:, None], in_=wt[:, j : j + 1], in_offset=None,
                        out_offset=bass.IndirectOffsetOnAxis(ap=act[:, j : j + 1], axis=0))

            # ============ Phase C: reduce spreads, AllReduce, recip =========
            for ci, (tbl, row) in enumerate([(spread_r, 0), (spread_c, 1)]):
                tv = tbl[:].rearrange("(c p s k) -> c p s k", c=RC, p=128, k=S)
                for ch in range(RC):
                    ld = red.tile([128, RS, S], F32, tag="spread_ld")
                    nc.sync.dma_start(out=ld[:], in_=tv[ch])
                    sm = red.tile([128, RS], F32, tag="spread_sum")
                    nc.vector.tensor_reduce(out=sm[:], in_=ld[:], axis=AX.X, op=ALU.add)
                    nc.sync.dma_start(
                        out=psum_in[row, ch * 128 * RS : (ch + 1) * 128 * RS]
                        .rearrange("(p s) -> p s", p=128),
                        in_=sm[:])

            nc.gpsimd.collective_compute(
                "AllReduce", ALU.add, replica_groups=groups,
                ins=[psum_in[:, :]], outs=[psum_out[:, :]])

            for row, rtab in ((0, r_row), (1, r_col)):
                for ch in range(RC):
                    sm = red.tile([128, RS], F32, tag="rsum")
                    nc.sync.dma_start(
                        out=sm[:],
                        in_=psum_out[row, ch * 128 * RS : (ch + 1) * 128 * RS]
                        .rearrange("(p s) -> p s", p=128))
                    # r = 0.5 / sum  ==  1 / (2*sum); +tiny keeps empty rows finite
                    nc.vector.tensor_scalar(out=sm[:], in0=sm[:], scalar1=2.0,
                                            scalar2=1e-30, op0=ALU.mult, op1=ALU.add)
                    rr = red.tile([128, RS], F32, tag="rrec")
                    nc.vector.reciprocal(out=rr[:], in_=sm[:])
                    nc.sync.dma_start(
                        out=rtab[ch * 128 * RS : (ch + 1) * 128 * RS]
                        .rearrange("(p s) -> p s", p=128),
                        in_=rr[:])

            # ================= Phase D: normalize + outputs =================
            wr_v = w_row[:].rearrange("(t p j) -> t p j", t=NT, p=128)
            wc_v = w_col[:].rearrange("(t p j) -> t p j", t=NT, p=128)
            for t in range(NT):
                st = edge.tile([128, EJ], I32, tag="st2")
                dt_ = edge.tile([128, EJ], I32, tag="dt2")
                nc.sync.dma_start(out=st[:], in_=src_v[t])
                nc.sync.dma_start(out=dt_[:], in_=dst_v[t])
                rr = gath.tile([128, EJ], F32, tag="rr")
                for j in range(EJ):
                    nc.gpsimd.indirect_dma_start(
                        out=rr[:, j : j + 1], out_offset=None, in_=r_row[:, None],
                        in_offset=bass.IndirectOffsetOnAxis(ap=st[:, j : j + 1], axis=0))
                rc_ = gath.tile([128, EJ], F32, tag="rc")
                for j in range(EJ):
                    nc.gpsimd.indirect_dma_start(
                        out=rc_[:, j : j + 1], out_offset=None, in_=r_col[:, None],
                        in_offset=bass.IndirectOffsetOnAxis(ap=dt_[:, j : j + 1], axis=0))
                wt = w_all[:, t * EJ : (t + 1) * EJ]
                wo = work.tile([128, EJ], F32, tag="wo")
                nc.vector.tensor_tensor(out=wo[:], in0=wt, in1=rr[:], op=ALU.mult)
                nc.sync.dma_start(out=wr_v[t], in_=wo[:])
                wo2 = work.tile([128, EJ], F32, tag="wo2")
                nc.vector.tensor_tensor(out=wo2[:], in0=wt, in1=rc_[:], op=ALU.mult)
                nc.sync.dma_start(out=wc_v[t], in_=wo2[:])

            if debug:
                nc.sync.dma_start(out=dbg_T[:, :], in_=T_full[:256, :])
                nc.sync.dma_start(
                    out=dbg_w[:].rearrange("(p j) -> p j", p=128),
                    in_=w_all[:, :EJ])
                nc.sync.dma_start(out=dbg_ps[:, :], in_=psum_in[:, :256])
                nc.sync.dma_start(out=dbg_po[:, :], in_=psum_out[:, :256])
                nc.sync.dma_start(out=dbg_r[:], in_=r_row[:256])

    nc.compile()
    return nc, c


def _occ_ranks(x):
    """occurrence rank of each element within its equal-value group."""
    order = np.argsort(x, kind="stable")
    xs = x[order]
    starts = np.flatnonzero(np.r_[True, xs[1:] != xs[:-1]])
    counts = np.diff(np.r_[starts, len(xs)])
    r = np.arange(len(xs)) - np.repeat(starts, counts)
    out = np.empty(len(xs), dtype=np.int64)
    out[order] = r
    return out


def make_in_maps(cfg, features, vertices, W1, b1, W2, b2, edges):
    c = derive(cfg)
    NSH, EPC, EPAD, S, NPAD = c["NSH"], c["EPC"], c["EPAD"], c["S"], c["NPAD"]
    trash = (NPAD - 1) * S  # slots on a padded node that is never read
    in_maps = []
    for cc in range(NCORES):
        sl = slice(cc * EPC, (cc + 1) * EPC)
        s_ = np.ascontiguousarray(edges[0, sl])
        d_ = np.ascontiguousarray(edges[1, sl])
        occ_s = _occ_ranks(s_)
        occ_d = _occ_ranks(d_)
        mo = max(occ_s.max(), occ_d.max())
        assert mo < S, f"need S > {mo}"
        ar = (s_.astype(np.int64) * S + occ_s).astype(np.int32)
        ac = (d_.astype(np.int64) * S + occ_d).astype(np.int32)
        npad = EPAD - EPC
        if npad:
            pad_i = np.zeros(npad, np.int32)
            pad_a = np.full(npad, trash, np.int32)
            s_ = np.r_[s_, pad_i]
            d_ = np.r_[d_, pad_i]
            ar = np.r_[ar, pad_a]
            ac = np.r_[ac, pad_a]
        nsl = slice(cc * NSH, (cc + 1) * NSH)
        in_maps.append({
            "feat": np.ascontiguousarray(features[nsl]),
            "vert": np.ascontiguousarray(vertices[nsl]),
            "W1": W1, "b1": b1, "W2": W2, "b2": b2,
            "src": s_, "dst": d_, "a_row": ar, "a_col": ac,
        })
    return in_maps


_BUILT = None


def kernel(features, vertices, W1, b1, W2, b2, edges):
    global _BUILT
    from concourse.bass_utils import run_bass_kernel_spmd

    features = np.asarray(features, np.float32)
    vertices = np.asarray(vertices, np.float32)
    W1 = np.asarray(W1, np.float32)
    b1 = np.asarray(b1, np.float32)
    W2 = np.asarray(W2, np.float32)
    b2 = np.asarray(b2, np.float32)
    edges = np.asarray(edges, np.int32)

    if _BUILT is None:
        _BUILT = build_kernel(FULL_CFG)
    nc, c = _BUILT
    in_maps = make_in_maps(FULL_CFG, features, vertices, W1, b1, W2, b2, edges)
    res = run_bass_kernel_spmd(nc, in_maps, list(range(NCORES))).results

    EPC = c["EPC"]
    wr = np.concatenate([np.asarray(r["w_row"])[:EPC] for r in res])
    wc = np.concatenate([np.asarray(r["w_col"])[:EPC] for r in res])
    values = np.concatenate([wr, wc])
    rows = np.concatenate([edges[0], edges[1]])
    cols = np.concatenate([edges[1], edges[0]])
    indices = np.stack([rows, cols]).astype(np.int32)
    return indices, values
